# revision 40
# baseline (speedup 1.0000x reference)
"""Trainium2 Bass kernel for nn_MultiHeadAttention (head-axis softmax quirk).

Strategy (8 NeuronCores):
  - Data-parallel over batch (2) x query-rows (4-way) => each core owns 512 q rows.
  - K/V projections sharded within each 4-core batch group. The gathers are
    split into 4 key-block sub-AllGathers each (kT_j / v_j, j = key-block),
    issued interleaved as soon as their producing projection output lands, with
    Shared-space outputs (fast HBM-HBM path). Attention chunks are processed
    grouped by key-block j so chunk-group j only waits on sub-gathers j --
    the collectives pipeline behind projection + attention compute.
  - Attention in scores^T layout [k, q]; softmax over HEADS: 2-head score
    waves -> one exp (ScalarE) per wave; identity-matmul head-sum accumulation
    grouped after the waves; normalize split DVE (12 heads) / GpSimd (4).
  - ctx: head-pairs 0-1 accumulate in 2 resident PSUM banks across the whole
    k-loop (PE start/stop); head-pairs 2-7 evacuate per chunk via DVE.
  - Output projection + residual + LayerNorm fused on-chip.
"""

import numpy as np
import ml_dtypes

D = 1024
H = 16
DK = 64
P = 128
M = 512  # q rows per core
SK = 2048  # k length per batch
G = 4  # cores per batch group
KC = SK // P
EC = D // P
HP = H // 2
LN_EPS = 1e-5

_CACHE = {}


def _build():
    from contextlib import ExitStack

    import concourse.bass as bass
    import concourse.mybir as mybir
    import concourse.tile as tile
    from concourse import bacc
    from concourse.masks import make_identity

    f32 = mybir.dt.float32
    bf16 = mybir.dt.bfloat16
    AF = mybir.ActivationFunctionType
    OP = mybir.AluOpType
    AX = mybir.AxisListType

    nc = bacc.Bacc("TRN2", target_bir_lowering=False, debug=False, num_devices=8)

    xqT = nc.dram_tensor("xqT", [D, M], bf16, kind="ExternalInput").ap()
    xkT = nc.dram_tensor("xkT", [D, M], bf16, kind="ExternalInput").ap()
    xvT = nc.dram_tensor("xvT", [D, M], bf16, kind="ExternalInput").ap()
    xres = nc.dram_tensor("xres", [M, D], f32, kind="ExternalInput").ap()
    wq = nc.dram_tensor("wq", [D, D], bf16, kind="ExternalInput").ap()
    wk = nc.dram_tensor("wk", [D, D], bf16, kind="ExternalInput").ap()
    wv = nc.dram_tensor("wv", [D, D], bf16, kind="ExternalInput").ap()
    wo = nc.dram_tensor("wo", [D, D], bf16, kind="ExternalInput").ap()
    gam = nc.dram_tensor("gam", [P, D], f32, kind="ExternalInput").ap()
    bet = nc.dram_tensor("bet", [P, D], f32, kind="ExternalInput").ap()
    bar = nc.dram_tensor("bar", [P, 64], bf16, kind="ExternalInput").ap()
    out = nc.dram_tensor("out", [M, D], f32, kind="ExternalOutput").ap()

    with tile.TileContext(nc) as tc:
        with ExitStack() as ctx:
            const = ctx.enter_context(tc.tile_pool(name="const", bufs=1))
            wpool = ctx.enter_context(tc.tile_pool(name="w", bufs=2))
            xtp = ctx.enter_context(tc.tile_pool(name="xt", bufs=2))
            pevac = ctx.enter_context(tc.tile_pool(name="pevac", bufs=3))
            kvstr = ctx.enter_context(tc.tile_pool(name="kvstr", bufs=3))
            expp = ctx.enter_context(tc.tile_pool(name="expp", bufs=3))
            smal = ctx.enter_context(tc.tile_pool(name="smal", bufs=3))
            resp = ctx.enter_context(tc.tile_pool(name="resp", bufs=2))
            lnp = ctx.enter_context(tc.tile_pool(name="lnp", bufs=2))
            psum = ctx.enter_context(tc.tile_pool(name="psum", bufs=2, space="PSUM"))
            dram = ctx.enter_context(tc.tile_pool(name="dram", bufs=1, space="DRAM"))

            # persistent tiles
            id_bf = const.tile([P, P], bf16)
            make_identity(nc, id_bf[:])
            ctx_sb = const.tile([P, HP, M], f32)
            qT_sb = const.tile([P, EC, M], bf16)
            gam_sb = const.tile([P, D], f32)
            nc.sync.dma_start(gam_sb[:], gam[:])
            bet_sb = const.tile([P, D], f32)
            nc.sync.dma_start(bet_sb[:], bet[:])

            # per-key-block collective buffers (j = key block of this core's
            # own 512-key slice; gathered slot g = group rank)
            # merged K^T+V collective buffer per key-block j:
            #   rows 0..1023  = kT block j  (row = o*128+p proj-dim, col = key)
            #   rows 1024..2047 = v block j ([128 keys, 1024] flat row-major)
            kv_in = [dram.tile([2 * D, P], bf16, name=f"kv_in{j}") for j in range(G)]
            kv_out = [
                dram.tile([G * 2 * D, P], bf16, name=f"kv_out{j}") for j in range(G)
            ]
            rg = [[0, 1, 2, 3], [4, 5, 6, 7]]

            # ---- Phase A: K^T projection (waves of 2 ec) ----
            wk_sb = wpool.tile([P, EC, D], bf16, tag="w")
            nc.sync.dma_start(wk_sb[:], wk.rearrange("(o p) e -> p o e", p=P))
            xk_sb = xtp.tile([P, EC, M], bf16, tag="xt")
            nc.sync.dma_start(xk_sb[:], xkT.rearrange("(o p) q -> p o q", p=P))

            kin_views = [
                t.rearrange("(t o p) k -> t p o k", t=2, p=P)[0] for t in kv_in
            ]
            for w0 in range(0, EC, 2):
                ps = psum.tile([P, 2, M], f32, tag="sc")
                for wj in range(2):
                    for dc in range(EC):
                        nc.tensor.matmul(
                            ps[:, wj, :],
                            lhsT=wk_sb[:, dc, (w0 + wj) * P : (w0 + wj + 1) * P],
                            rhs=xk_sb[:, dc, :],
                            start=(dc == 0),
                            stop=(dc == EC - 1),
                        )
                ev = pevac.tile([P, 2, M], bf16, tag="pe")
                nc.vector.tensor_copy(ev[:], ps[:])
                for j in range(G):
                    nc.sync.dma_start(
                        kin_views[j][:, w0 : w0 + 2, :], ev[:, :, j * P : (j + 1) * P]
                    )

            # ---- V projection (key-block-major) + interleaved sub-gathers ----
            # kT_j data is complete after the full K-proj; v_j after V-proj
            # block j. Interleave kT_j/v_j issue so the CC engine pipelines
            # chunk-group j's data earliest-first.
            wv_sb = wpool.tile([P, EC, D], bf16, tag="w")
            nc.sync.dma_start(wv_sb[:], wv.rearrange("(o p) e -> p o e", p=P))
            xv_sb = xtp.tile([P, EC, M], bf16, tag="xt")
            nc.sync.dma_start(xv_sb[:], xvT.rearrange("(o p) q -> p o q", p=P))

            vin_views = [
                t.rearrange("(t k o) c -> t k (o c)", t=2, k=P)[1] for t in kv_in
            ]
            for j in range(G):
                ps = psum.tile([P, 2, M], f32, tag="sc")
                for eh in range(2):
                    for dc in range(EC):
                        nc.tensor.matmul(
                            ps[:, eh, :],
                            lhsT=xv_sb[:, dc, j * P : (j + 1) * P],
                            rhs=wv_sb[:, dc, eh * M : (eh + 1) * M],
                            start=(dc == 0),
                            stop=(dc == EC - 1),
                        )
                ev = pevac.tile([P, 2, M], bf16, tag="pe")
                nc.vector.tensor_copy(ev[:], ps[:])
                for eh in range(2):
                    nc.sync.dma_start(
                        vin_views[j][:, eh * M : (eh + 1) * M], ev[:, eh, :]
                    )
                nc.gpsimd.collective_compute(
                    "AllGather",
                    OP.bypass,
                    replica_groups=rg,
                    ins=[kv_in[j].opt()],
                    outs=[kv_out[j].opt()],
                )

            # ---- Phase B: Q^T projection (local) ----
            wq_sb = wpool.tile([P, EC, D], bf16, tag="w")
            nc.sync.dma_start(wq_sb[:], wq.rearrange("(o p) e -> p o e", p=P))
            xq_sb = xtp.tile([P, EC, M], bf16, tag="xt")
            nc.sync.dma_start(xq_sb[:], xqT.rearrange("(o p) q -> p o q", p=P))
            for w0 in range(0, EC, 2):
                ps = psum.tile([P, 2, M], f32, tag="sc")
                for wj in range(2):
                    for dc in range(EC):
                        nc.tensor.matmul(
                            ps[:, wj, :],
                            lhsT=wq_sb[:, dc, (w0 + wj) * P : (w0 + wj + 1) * P],
                            rhs=xq_sb[:, dc, :],
                            start=(dc == 0),
                            stop=(dc == EC - 1),
                        )
                nc.vector.tensor_copy(qT_sb[:, w0 : w0 + 2, :], ps[:])

            # (load W_O early; DMA overlaps the attention loop)
            wo_sb = wpool.tile([P, EC, D], bf16, tag="w")
            nc.sync.dma_start(wo_sb[:], wo.rearrange("(o p) e -> p o e", p=P))

            # ---- Phase C: attention, chunk-groups by key-block j ----
            kT_views = [
                t.rearrange("(g t o p) k -> t g p o k", g=G, t=2, p=P)[0]
                for t in kv_out
            ]
            v_views = [
                t.rearrange("(g t k o) c -> t g k (o c)", g=G, t=2, k=P)[1]
                for t in kv_out
            ]

            order = [g * G + j for j in range(G) for g in range(G)]

            def emit_ctx_pair(vt, et, ci, hp):
                # one head-pair's context matmuls + DVE evacuation
                cps = psum.tile([P, M], f32, tag="cps", name="cps")
                nc.tensor.matmul(
                    cps[0:DK, :],
                    lhsT=vt[:, (2 * hp) * DK : (2 * hp + 1) * DK],
                    rhs=et[:, 2 * hp, :],
                    start=True,
                    stop=True,
                    tile_position=(0, 0),
                )
                nc.tensor.matmul(
                    cps[DK:P, :],
                    lhsT=vt[:, (2 * hp + 1) * DK : (2 * hp + 2) * DK],
                    rhs=et[:, 2 * hp + 1, :],
                    start=True,
                    stop=True,
                    tile_position=(0, 64),
                )
                if ci == 0:
                    nc.vector.tensor_copy(ctx_sb[:, hp, :], cps[:])
                else:
                    nc.vector.tensor_tensor(
                        ctx_sb[:, hp, :], ctx_sb[:, hp, :], cps[:], OP.add
                    )

            def emit_ctx(vt, et, ci):
                for hp in range(HP):
                    emit_ctx_pair(vt, et, ci, hp)

            prev = None
            for ci, kc in enumerate(order):
                g, j = divmod(kc, G)
                kt = kvstr.tile([P, EC, P], bf16, tag="kt")
                nc.sync.dma_start(kt[:], kT_views[j][g])
                vt = kvstr.tile([P, D], bf16, tag="vt")
                nc.sync.dma_start(vt[:], v_views[j][g])

                et = expp.tile([P, H, M], bf16, tag="exp")
                dps = psum.tile([P, M], f32, tag="dps")
                for w in range(H // 2):
                    ps = psum.tile([P, 2, M], f32, tag="sc")
                    for wj in range(2):
                        h = 2 * w + wj
                        hp, half = divmod(h, 2)
                        pb = half * DK
                        nc.tensor.matmul(
                            ps[:, wj, :],
                            lhsT=kt[pb : pb + DK, hp, :],
                            rhs=qT_sb[pb : pb + DK, hp, :],
                            start=True,
                            stop=True,
                        )
                    nc.scalar.activation(
                        et[:, 2 * w : 2 * w + 2, :], ps[:], AF.Exp, scale=0.125
                    )
                    if w >= 2:
                        # head-sum identity matmuls of wave w-2 (exp complete:
                        # sc bufs=2 already forced that) fill the PE gap while
                        # ScalarE paces the score waves
                        for hh in (2 * (w - 2), 2 * (w - 2) + 1):
                            nc.tensor.matmul(
                                dps[:],
                                lhsT=id_bf[:],
                                rhs=et[:, hh, :],
                                start=(hh == 0),
                                stop=False,
                                skip_group_check=True,
                            )
                    if w >= 6 and prev is not None:
                        # previous chunk's ctx pairs also fill late-wave gaps
                        # (its normalize finished early in this score phase)
                        emit_ctx_pair(prev[0], prev[1], prev[2], w - 6)

                # rest of the previous chunk's ctx matmuls
                if prev is not None:
                    for hp in range(2, HP):
                        emit_ctx_pair(prev[0], prev[1], prev[2], hp)

                for hh in range(H - 4, H):
                    nc.tensor.matmul(
                        dps[:],
                        lhsT=id_bf[:],
                        rhs=et[:, hh, :],
                        start=False,
                        stop=(hh == H - 1),
                        skip_group_check=True,
                    )
                dsb = smal.tile([P, M], f32, tag="dsb")
                nc.scalar.copy(dsb[:], dps[:])
                rf = smal.tile([P, M], f32, tag="rf")
                nc.vector.reciprocal_approx_fast(rf[:], dsb[:])
                rb = smal.tile([P, M], bf16, tag="rb")
                nc.vector.tensor_copy(rb[:], rf[:])

                # normalize: DVE heads 0-11, GpSimd heads 12-15
                nc.vector.tensor_tensor(
                    et[:, :12, :],
                    et[:, :12, :],
                    rb[:, None, :].to_broadcast((P, 12, M)),
                    OP.mult,
                )
                nc.gpsimd.tensor_tensor(
                    et[:, 12:, :],
                    et[:, 12:, :],
                    rb[:, None, :].to_broadcast((P, H - 12, M)),
                    OP.mult,
                )
                prev = (vt, et, ci)

            emit_ctx(*prev)

            # ---- Phase D: output projection + residual + LayerNorm ----
            ctx_bf = const.tile([P, HP, M], bf16)
            nc.vector.tensor_copy(ctx_bf[:], ctx_sb[:])
            res_view = xres.rearrange("(o p) e -> o p e", p=P)
            out_view = out.rearrange("(o p) e -> o p e", p=P)
            for qc in range(M // P):
                rest = resp.tile([P, D], f32, tag="res")
                nc.sync.dma_start(rest[:], res_view[qc])
                xsb = lnp.tile([P, D], f32, tag="x")
                ps = psum.tile([P, 2, M], f32, tag="sc")
                for eh in range(2):
                    for vc in range(EC):
                        nc.tensor.matmul(
                            ps[:, eh, :],
                            lhsT=ctx_bf[:, vc, qc * P : (qc + 1) * P],
                            rhs=wo_sb[:, vc, eh * M : (eh + 1) * M],
                            start=(vc == 0),
                            stop=(vc == EC - 1),
                        )
                for eh in range(2):
                    nc.vector.tensor_tensor(
                        xsb[:, eh * M : (eh + 1) * M],
                        ps[:, eh, :],
                        rest[:, eh * M : (eh + 1) * M],
                        OP.add,
                    )

                # LN stats on ScalarE (DVE is the tail bottleneck):
                # copy-with-accum -> sum(x); center via per-partition bias;
                # square-with-accum -> sum((x-mu)^2)
                sq = lnp.tile([P, D], f32, tag="sq", bufs=1)
                mu_r = smal.tile([P, 1], f32, tag="mu")
                nc.scalar.activation(sq[:], xsb[:], AF.Identity, accum_out=mu_r[:])
                mu_neg = smal.tile([P, 1], f32, tag="mu2")
                nc.vector.tensor_scalar_mul(mu_neg[:], mu_r[:], -1.0 / D)
                xc = lnp.tile([P, D], f32, tag="xc")
                nc.scalar.activation(xc[:], xsb[:], AF.Identity, bias=mu_neg[:])
                var_r = smal.tile([P, 1], f32, tag="var")
                nc.scalar.activation(sq[:], xc[:], AF.Square, accum_out=var_r[:])
                veps = smal.tile([P, 1], f32, tag="veps")
                nc.vector.tensor_scalar(
                    veps[:], var_r[:], 1.0 / D, LN_EPS, OP.mult, OP.add
                )
                iv2 = smal.tile([P, 1], f32, tag="iv2")
                nc.vector.reciprocal_approx_fast(iv2[:], veps[:])
                inv = smal.tile([P, 1], f32, tag="inv")
                nc.scalar.activation(inv[:], iv2[:], AF.Sqrt)
                nc.vector.scalar_tensor_tensor(
                    xc[:], xc[:], inv[:], gam_sb[:], OP.mult, OP.mult
                )
                ot = lnp.tile([P, D], f32, tag="ot")
                nc.vector.tensor_tensor(ot[:], xc[:], bet_sb[:], OP.add)
                nc.sync.dma_start(out_view[qc], ot[:])

    nc.compile()
    return nc


def _get_nc():
    if "nc" not in _CACHE:
        _CACHE["nc"] = _build()
    return _CACHE["nc"]


def _in_maps(input_Q, input_K, input_V, W_Q, W_K, W_V, W_O, ln_gamma, ln_beta):
    bf = ml_dtypes.bfloat16
    f32 = np.float32
    Q_ = np.asarray(input_Q, dtype=f32)
    K_ = np.asarray(input_K, dtype=f32)
    V_ = np.asarray(input_V, dtype=f32)
    wq_b = np.asarray(W_Q, dtype=f32).astype(bf)
    wk_b = np.asarray(W_K, dtype=f32).astype(bf)
    wv_b = np.asarray(W_V, dtype=f32).astype(bf)
    wo_b = np.asarray(W_O, dtype=f32).astype(bf)
    gam_b = np.ascontiguousarray(
        np.broadcast_to(np.asarray(ln_gamma, dtype=f32), (P, D))
    )
    bet_b = np.ascontiguousarray(
        np.broadcast_to(np.asarray(ln_beta, dtype=f32), (P, D))
    )
    maps = []
    for c in range(8):
        b, r = divmod(c, G)
        sl = slice(r * M, (r + 1) * M)
        maps.append(
            {
                "xqT": np.ascontiguousarray(Q_[b, sl].T).astype(bf),
                "xkT": np.ascontiguousarray(K_[b, sl].T).astype(bf),
                "xvT": np.ascontiguousarray(V_[b, sl].T).astype(bf),
                "xres": np.ascontiguousarray(Q_[b, sl]),
                "wq": wq_b,
                "wk": wk_b,
                "wv": wv_b,
                "wo": wo_b,
                "gam": gam_b,
                "bet": bet_b,
                "bar": np.zeros((P, 64), dtype=bf),
            }
        )
    return maps


def _assemble(results):
    B = 2
    out = np.empty((B, SK, D), np.float32)
    for c in range(8):
        b, r = divmod(c, G)
        out[b, r * M : (r + 1) * M] = results[c]["out"]
    return out


def run_traced(trace=False, **inputs):
    """Run on HW; returns (output, BassKernelResults)."""
    from concourse.bass_utils import run_bass_kernel_spmd

    nc = _get_nc()
    maps = _in_maps(**inputs)
    res = run_bass_kernel_spmd(nc, maps, list(range(8)), trace=trace)
    return _assemble(res.results), res


def kernel(**inputs) -> np.ndarray:
    out, _ = run_traced(trace=False, **inputs)
    return out
